# revision 6
# baseline (speedup 1.0000x reference)
"""Trainium2 Bass kernel for DeformablePatchSampler2d (v3).

out[n, m, c, i, j] = bilinear_sample(x[n, c], row=RY[m, j], col=CX[m, i])

Sampling grid is batch/channel-invariant and known on the host from
`offset`; windows/weights are baked in at build time. Data-parallel over
batch N=8 across 8 cores.

v3 structure (per core):
  - 4 band-PAIRS: partition half s holds band b = p + 4*s (64 channels
    each), so every compute op runs 128 partitions wide with no
    partition shifts.
  - full-width row loads: rows [r0_b, r0_b+20) x 384 cols, one 30KB
    descriptor per (channel, band) -> 512 descriptors total instead of
    the v2 design's 20480 x 504B.
  - one ACT copy per pair converts the f32 band to fp16 (2-byte dtype
    unlocks the DVE 2x mode); all tensor ops and the output store are
    fp16 (rel-err gate is 2e-2; fp16 sampling lands ~1e-3).
  - stage 1 (row taps) reads the band tile directly per slot (no
    gather copies); stage 2 (col taps) is merged across all 8 slots
    with weight APs shaped [slot][0,j-bcast][i-packed] so DVE runs at
    2x. Slots are sorted by tap count so tap-k ops cover a prefix.
  - outputs are written [band, c, slot, 16*16] so each store descriptor
    is 4KB; the host unpermutes and upcasts.
"""
import numpy as np

_P = 16
_NPH = _NPW = 8
_M = 64
_H = _W = 384
_C = 64
_N = 8
_RW = 20            # rows per band tile
_RT_MAX = 4         # row-tap slots in the weight layout
_CT_MAX = 3         # col-tap slots
_WSLOT = _RT_MAX * 16 + _CT_MAX * 16   # 112 weight floats per slot
_WPAIR = 8 * _WSLOT                    # 896 per pair
_XDT = "f32"        # dtype of the device-side x tensor: "f32" | "f16"

# engine assignment: pool takes ~15 DVE-us of work (it runs ~2x slower)
_S1_POOL_PAIRS = ()        # s1 mult/add pairs on Pool
_S1A_POOL_PAIRS = (0, 1)   # s1 adds on Pool
_S2_POOL_PAIRS = (2, 3)    # s2 entirely on Pool


def _precompute(offset: np.ndarray):
    """Window origins + 3-tap weights, f32 coord math mirroring the reference."""
    offset = offset.astype(np.float32)
    one, half = np.float32(1.0), np.float32(0.5)
    ch = np.linspace(0.0, float(_H), _NPH + 4).astype(np.float32)[2:-2]
    cw = np.linspace(0.0, float(_W), _NPW + 4).astype(np.float32)[2:-2]
    rel = np.arange(_P, dtype=np.float32) - np.float32(_P // 2)
    a = np.arange(_M) // _NPW
    b = np.arange(_M) % _NPW
    hc = ch[a][:, None] + rel[None, :]
    wcen = cw[b][:, None] + rel[None, :]
    gx = (np.float32(2.0) * hc / np.float32(_H - 1) - one) + offset[:, 0:1]
    gy = (np.float32(2.0) * wcen / np.float32(_W - 1) - one) + offset[:, 1:2]
    CX = (((gx + one) * np.float32(_W) - one) * half).astype(np.float64)  # (M,16) cols, dim i
    RY = (((gy + one) * np.float32(_H) - one) * half).astype(np.float64)  # (M,16) rows, dim j

    r0 = np.floor(RY[:, 0]).astype(np.int64)
    c0 = np.floor(CX[:, 0]).astype(np.int64)
    t_r = RY - (r0[:, None] + np.arange(_P)[None, :])
    t_c = CX - (c0[:, None] + np.arange(_P)[None, :])
    assert (t_r >= 0).all() and (t_r < 2).all()
    assert (t_c >= 0).all() and (t_c < 2).all()
    assert r0.min() >= 0 and (r0 + 17).max() <= _H - 1
    assert c0.min() >= 0 and (c0 + 17).max() <= _W - 1

    def taps(t):
        w0 = np.maximum(0.0, 1.0 - t)
        w2 = np.maximum(0.0, t - 1.0)
        return np.stack([w0, 1.0 - w0 - w2, w2], axis=-1).astype(np.float32)

    wr = taps(t_r)  # (M, 16, 3) applies to j (rows)
    wc = taps(t_c)  # (M, 16, 3) applies to i (cols)
    nt_r = np.where(np.abs(wr[:, :, 2]).max(axis=1) > 0, 3, 2)
    nt_c = np.where(np.abs(wc[:, :, 2]).max(axis=1) > 0, 3, 2)
    return r0, c0, wr, wc, nt_r, nt_c


def _plan(offset: np.ndarray):
    r0, c0, wr, wc, nt_r, nt_c = _precompute(offset)
    mw_of = np.arange(_M) % _NPW
    band_r0 = np.array([r0[mw_of == b].min() for b in range(8)])
    assert all(r0[m] - band_r0[mw_of[m]] <= 1 for m in range(_M))
    assert band_r0.max() + _RW <= _H

    w_all = np.zeros((128, 4 * _WPAIR), dtype=np.float32)
    pairs = []
    for p in range(4):
        bands = (p, p + 4)
        slots = []
        for mh in range(8):
            ms = [mh * 8 + bands[s] for s in range(2)]
            rho = [int(r0[m] - band_r0[mw_of[m]]) for m in ms]
            sig = [int(c0[m]) for m in ms]
            rho0, sig0 = min(rho), min(sig)
            rt = max(rho[s] - rho0 + int(nt_r[ms[s]]) for s in range(2))
            ct = max(sig[s] - sig0 + int(nt_c[ms[s]]) for s in range(2))
            assert rt <= _RT_MAX and ct <= _CT_MAX
            assert rho0 + 15 + rt <= _RW
            assert sig0 + 15 + ct <= _W
            wrs = np.zeros((2, _RT_MAX, 16), dtype=np.float32)
            wcs = np.zeros((2, _CT_MAX, 16), dtype=np.float32)
            for s in range(2):
                rs, cs = rho[s] - rho0, sig[s] - sig0
                wrs[s, rs:rs + 3] = wr[ms[s]].T
                wcs[s, cs:cs + 3] = wc[ms[s]].T
            slots.append(dict(mh=mh, rho0=rho0, sig0=sig0, rt=rt, ct=ct,
                              wrs=wrs, wcs=wcs))
        # sort so tap-k ops cover a slot prefix
        slots.sort(key=lambda sl: (sl["rt"], sl["ct"]), reverse=True)
        for pos, sl in enumerate(slots):
            base = p * _WPAIR + pos * _WSLOT
            for s in range(2):
                rows = slice(s * 64, (s + 1) * 64)
                w_all[rows, base:base + _RT_MAX * 16] = \
                    sl["wrs"][s].reshape(-1)[None, :]
                w_all[rows, base + _RT_MAX * 16:base + _WSLOT] = \
                    sl["wcs"][s].reshape(-1)[None, :]
        pairs.append(dict(p=p, bands=bands,
                          r0=[int(band_r0[b]) for b in bands],
                          slots=slots))
    return pairs, w_all


def _build(pairs, xdt_name):
    import concourse.bacc as bacc
    import concourse.mybir as mybir
    from concourse.bass import AP
    from concourse.tile import TileContext

    f32 = mybir.dt.float32
    f16 = mybir.dt.float16
    xdt = f32 if xdt_name == "f32" else f16
    mult = mybir.AluOpType.mult
    add = mybir.AluOpType.add

    nc = bacc.Bacc("TRN2", target_bir_lowering=False)
    x_n = nc.dram_tensor("x_n", (_C, _H, _W), xdt, kind="ExternalInput")
    w_d = nc.dram_tensor("w_all", (128, 4 * _WPAIR), f16, kind="ExternalInput")
    out_d = nc.dram_tensor("out_d", (8, _C, 8, 256), f16, kind="ExternalOutput")

    def sub_ap(base_ap, extra_off, free_dims):
        return AP(base_ap.tensor, base_ap.offset + extra_off,
                  [list(base_ap.ap[0])] + [list(d) for d in free_dims])

    ROWLEN = _RW * _W  # 7680 elements per partition per band

    with TileContext(nc) as tc:
        with tc.tile_pool(name="wpool", bufs=1) as wpool, \
             tc.tile_pool(name="fpool", bufs=2) as fpool, \
             tc.tile_pool(name="bpool", bufs=2) as bpool, \
             tc.tile_pool(name="tpool", bufs=2) as tpool, \
             tc.tile_pool(name="mpool", bufs=2) as mpool, \
             tc.tile_pool(name="opool", bufs=2) as opool, \
             tc.tile_pool(name="npool", bufs=2) as npool:
            W_sb = wpool.tile([128, 4 * _WPAIR], f16)
            nc.scalar.dma_start(out=W_sb[:], in_=w_d[:])
            wb = W_sb[:]

            def emit_load(pair):
                # full-width rows: one 30KB descriptor per (channel, band)
                F = fpool.tile([128, ROWLEN], xdt)
                for s in range(2):
                    src = AP(x_n[:].tensor, pair["r0"][s] * _W,
                             [[_H * _W, _C], [1, ROWLEN]])
                    nc.sync.dma_start(out=F[s * 64:(s + 1) * 64, :], in_=src)
                return F

            def emit_convert(F):
                if xdt_name == "f16":
                    return F
                B = bpool.tile([128, ROWLEN], f16)
                nc.scalar.copy(out=B[:], in_=F[:])
                return B

            def emit_store(pair, O):
                for s in range(2):
                    b = pair["bands"][s]
                    dst = AP(out_d[:].tensor, b * (_C * 8 * 256),
                             [[8 * 256, _C], [1, 8 * 256]])
                    nc.scalar.dma_start(out=dst, in_=O[s * 64:(s + 1) * 64, :])

            PREFETCH = 2
            ftiles = {i: emit_load(pairs[i]) for i in range(PREFETCH)}
            btiles = {0: emit_convert(ftiles[0])}
            pending_store = None
            for pi, pair in enumerate(pairs):
                p = pair["p"]
                slots = pair["slots"]
                bb = btiles.pop(pi)[:]
                ftiles.pop(pi)
                if pi + 1 < len(pairs):
                    btiles[pi + 1] = emit_convert(ftiles[pi + 1])
                if pi + PREFETCH < len(pairs):
                    ftiles[pi + PREFETCH] = emit_load(pairs[pi + PREFETCH])

                e1 = nc.gpsimd if p in _S1_POOL_PAIRS else nc.vector
                e1a = nc.gpsimd if p in _S1A_POOL_PAIRS else e1
                e2 = nc.gpsimd if p in _S2_POOL_PAIRS else nc.vector

                wpair = p * _WPAIR
                T = tpool.tile([128, 8 * 288], f16)
                tb = T[:]
                rtmax = max(sl["rt"] for sl in slots)
                ctmax = max(sl["ct"] for sl in slots)
                Mts = {}
                for k in range(rtmax):
                    nk = sum(1 for sl in slots if sl["rt"] > k)
                    if k > 0:
                        Mts[k] = mpool.tile([128, 8 * 288], f16, name=f"Mt{k}")
                    for pos, sl in enumerate(slots):
                        if sl["rt"] <= k:
                            break
                        src = sub_ap(bb, (sl["rho0"] + k) * _W + sl["sig0"],
                                     [[_W, 16], [1, 18]])
                        w_ap = sub_ap(wb, wpair + pos * _WSLOT + k * 16,
                                      [[1, 16], [0, 18]])
                        dstt = tb if k == 0 else Mts[k][:]
                        dst = sub_ap(dstt, pos * 288, [[18, 16], [1, 18]])
                        e1.tensor_tensor(out=dst, in0=src, in1=w_ap, op=mult)
                    if k > 0:
                        e1a.tensor_tensor(
                            out=sub_ap(tb, 0, [[1, nk * 288]]),
                            in0=sub_ap(tb, 0, [[1, nk * 288]]),
                            in1=sub_ap(Mts[k][:], 0, [[1, nk * 288]]),
                            op=add)
                O = opool.tile([128, 8 * 256], f16)
                ob = O[:]
                for ik in range(ctmax):
                    nk = sum(1 for sl in slots if sl["ct"] > ik)
                    in0 = sub_ap(tb, ik, [[288, nk], [18, 16], [1, 16]])
                    w_ap = sub_ap(wb, wpair + _RT_MAX * 16 + ik * 16,
                                  [[_WSLOT, nk], [0, 16], [1, 16]])
                    if ik == 0:
                        o_ap = sub_ap(ob, 0, [[256, nk], [16, 16], [1, 16]])
                        e2.tensor_tensor(out=o_ap, in0=in0, in1=w_ap, op=mult)
                    else:
                        MO = npool.tile([128, 8 * 256], f16)
                        m_ap = sub_ap(MO[:], 0, [[256, nk], [16, 16], [1, 16]])
                        e2.tensor_tensor(out=m_ap, in0=in0, in1=w_ap, op=mult)
                        e2.tensor_tensor(
                            out=sub_ap(ob, 0, [[1, nk * 256]]),
                            in0=sub_ap(ob, 0, [[1, nk * 256]]),
                            in1=sub_ap(MO[:], 0, [[1, nk * 256]]),
                            op=add)
                if pending_store is not None:
                    emit_store(*pending_store)
                pending_store = (pair, O)

            emit_store(*pending_store)
    nc.compile()
    return nc


def _prepare(offset):
    pairs, w_all = _plan(offset)
    nc = _build(pairs, _XDT)
    aux = dict(pairs=pairs, w_all=w_all.astype(np.float16))
    return nc, aux


def _run(nc, x, aux, **kwargs):
    from concourse.bass_utils import run_bass_kernel_spmd
    xdt = np.float32 if _XDT == "f32" else np.float16
    in_maps = [{"x_n": np.ascontiguousarray(x[n]).astype(xdt, copy=False),
                "w_all": aux["w_all"]}
               for n in range(_N)]
    return run_bass_kernel_spmd(nc, in_maps, core_ids=list(range(_N)), **kwargs)


def _postprocess(out_dev, pairs):
    """out_dev: (8 bands, C, 8 slotpos, 256) fp16 -> (M, C, 16, 16) f32."""
    out = np.empty((_M, _C, _P, _P), dtype=np.float32)
    for p in range(4):
        order = [sl["mh"] for sl in pairs[p]["slots"]]
        for s in range(2):
            b = p + 4 * s
            for pos, mh in enumerate(order):
                # device slot layout is [j][i]-major (keeps DVE last dims
                # packed); semantic output is [i][j]
                out[mh * 8 + b] = out_dev[b, :, pos].astype(
                    np.float32).reshape(_C, _P, _P).transpose(0, 2, 1)
    return out


def kernel(x: np.ndarray, offset: np.ndarray) -> np.ndarray:
    x = np.asarray(x, dtype=np.float32)
    offset = np.asarray(offset, dtype=np.float32)
    nc, aux = _prepare(offset)
    res = _run(nc, x, aux)
    return np.stack([_postprocess(res.results[n]["out_d"], aux["pairs"])
                     for n in range(_N)])


# revision 11
# speedup vs baseline: 1.7362x; 1.7362x over previous
"""Trainium2 Bass kernel for DeformablePatchSampler2d (v4).

out[n, m, c, i, j] = bilinear_sample(x[n, c], row=RY[m, j], col=CX[m, i])

Sampling grid is batch/channel-invariant and known on the host from
`offset`; windows/weights are baked in at build time. Data-parallel over
batch N=8 across 8 cores.

v4 structure (per core):
  - 4 band-PAIRS: partition half s holds band b = p + 4*s (64 channels
    each), so every compute op runs 128 partitions wide with no
    partition shifts.
  - x is uploaded as fp16 (host cast): the DMA fabric tops out at
    ~225 GB/s per core regardless of packet size, so halving the load
    bytes is the single biggest lever. Loads fetch only the per-band
    column span actually sampled (~266 of 384 cols). The rel-err gate
    is 2e-2; fp16 sampling lands ~5e-4.
  - stage 1 (row taps) reads the band tile directly per slot (no
    gather copies); stage 2 (col taps) is merged across all 8 slots
    with weight APs shaped [slot][0,j-bcast][i-packed] so DVE runs at
    2x. Slots are sorted by tap count so tap-k ops cover a prefix.
  - outputs are written [band, c, slot, 16*16] so each store descriptor
    is 4KB; the host unpermutes and upcasts.
"""
import numpy as np

_P = 16
_NPH = _NPW = 8
_M = 64
_H = _W = 384
_C = 64
_N = 8
_RW = 20            # rows per band tile
_RT_MAX = 4         # row-tap slots in the weight layout
_CT_MAX = 3         # col-tap slots
_WSLOT = _RT_MAX * 16 + _CT_MAX * 16   # 112 weight floats per slot
_WPAIR = 8 * _WSLOT                    # 896 per pair
_XDT = "f16"        # dtype of the device-side x tensor: "f32" | "f16"

# engine split: DVE gets all 2x-mode ops (s1 adds, s2) plus the first
# _S1_DVE_SLOTS[p] slots' s1 mults; Pool (no fast modes, but low per-op
# overhead) takes the remaining 1x-mode s1 mults.
_S1_DVE_SLOTS = (3, 2, 3, 2)


def _precompute(offset: np.ndarray):
    """Window origins + 3-tap weights, f32 coord math mirroring the reference."""
    offset = offset.astype(np.float32)
    one, half = np.float32(1.0), np.float32(0.5)
    ch = np.linspace(0.0, float(_H), _NPH + 4).astype(np.float32)[2:-2]
    cw = np.linspace(0.0, float(_W), _NPW + 4).astype(np.float32)[2:-2]
    rel = np.arange(_P, dtype=np.float32) - np.float32(_P // 2)
    a = np.arange(_M) // _NPW
    b = np.arange(_M) % _NPW
    hc = ch[a][:, None] + rel[None, :]
    wcen = cw[b][:, None] + rel[None, :]
    gx = (np.float32(2.0) * hc / np.float32(_H - 1) - one) + offset[:, 0:1]
    gy = (np.float32(2.0) * wcen / np.float32(_W - 1) - one) + offset[:, 1:2]
    CX = (((gx + one) * np.float32(_W) - one) * half).astype(np.float64)  # (M,16) cols, dim i
    RY = (((gy + one) * np.float32(_H) - one) * half).astype(np.float64)  # (M,16) rows, dim j

    r0 = np.floor(RY[:, 0]).astype(np.int64)
    c0 = np.floor(CX[:, 0]).astype(np.int64)
    t_r = RY - (r0[:, None] + np.arange(_P)[None, :])
    t_c = CX - (c0[:, None] + np.arange(_P)[None, :])
    assert (t_r >= 0).all() and (t_r < 2).all()
    assert (t_c >= 0).all() and (t_c < 2).all()
    assert r0.min() >= 0 and (r0 + 17).max() <= _H - 1
    assert c0.min() >= 0 and (c0 + 17).max() <= _W - 1

    def taps(t):
        w0 = np.maximum(0.0, 1.0 - t)
        w2 = np.maximum(0.0, t - 1.0)
        return np.stack([w0, 1.0 - w0 - w2, w2], axis=-1).astype(np.float32)

    wr = taps(t_r)  # (M, 16, 3) applies to j (rows)
    wc = taps(t_c)  # (M, 16, 3) applies to i (cols)
    nt_r = np.where(np.abs(wr[:, :, 2]).max(axis=1) > 0, 3, 2)
    nt_c = np.where(np.abs(wc[:, :, 2]).max(axis=1) > 0, 3, 2)
    return r0, c0, wr, wc, nt_r, nt_c


def _plan(offset: np.ndarray):
    r0, c0, wr, wc, nt_r, nt_c = _precompute(offset)
    mw_of = np.arange(_M) % _NPW
    band_r0 = np.array([r0[mw_of == b].min() for b in range(8)])
    band_c0 = np.array([c0[mw_of == b].min() for b in range(8)])
    span = int(max(c0[m] + 18 - band_c0[mw_of[m]] for m in range(_M)))
    span = (span + 1) & ~1  # keep 4-byte alignment for fp16 tiles
    assert all(r0[m] - band_r0[mw_of[m]] <= 1 for m in range(_M))
    assert band_r0.max() + _RW <= _H
    assert all(band_c0[b] + span <= _W for b in range(8))

    w_all = np.zeros((128, 4 * _WPAIR), dtype=np.float32)
    pairs = []
    for p in range(4):
        bands = (p, p + 4)
        slots = []
        for mh in range(8):
            ms = [mh * 8 + bands[s] for s in range(2)]
            rho = [int(r0[m] - band_r0[mw_of[m]]) for m in ms]
            sig = [int(c0[m] - band_c0[mw_of[m]]) for m in ms]
            rho0, sig0 = min(rho), min(sig)
            rt = max(rho[s] - rho0 + int(nt_r[ms[s]]) for s in range(2))
            ct = max(sig[s] - sig0 + int(nt_c[ms[s]]) for s in range(2))
            assert rt <= _RT_MAX and ct <= _CT_MAX
            assert rho0 + 15 + rt <= _RW
            assert sig0 + 15 + ct <= span
            wrs = np.zeros((2, _RT_MAX, 16), dtype=np.float32)
            wcs = np.zeros((2, _CT_MAX, 16), dtype=np.float32)
            for s in range(2):
                rs, cs = rho[s] - rho0, sig[s] - sig0
                wrs[s, rs:rs + 3] = wr[ms[s]].T
                wcs[s, cs:cs + 3] = wc[ms[s]].T
            slots.append(dict(mh=mh, rho0=rho0, sig0=sig0, rt=rt, ct=ct,
                              wrs=wrs, wcs=wcs))
        # sort so tap-k ops cover a slot prefix
        slots.sort(key=lambda sl: (sl["rt"], sl["ct"]), reverse=True)
        for pos, sl in enumerate(slots):
            base = p * _WPAIR + pos * _WSLOT
            for s in range(2):
                rows = slice(s * 64, (s + 1) * 64)
                w_all[rows, base:base + _RT_MAX * 16] = \
                    sl["wrs"][s].reshape(-1)[None, :]
                w_all[rows, base + _RT_MAX * 16:base + _WSLOT] = \
                    sl["wcs"][s].reshape(-1)[None, :]
        pairs.append(dict(p=p, bands=bands,
                          r0=[int(band_r0[b]) for b in bands],
                          c0=[int(band_c0[b]) for b in bands],
                          slots=slots))
    return pairs, span, w_all


def _build(pairs, span, xdt_name):
    import concourse.bacc as bacc
    import concourse.mybir as mybir
    from concourse.bass import AP
    from concourse.tile import TileContext

    f32 = mybir.dt.float32
    f16 = mybir.dt.float16
    xdt = f32 if xdt_name == "f32" else f16
    mult = mybir.AluOpType.mult
    add = mybir.AluOpType.add

    nc = bacc.Bacc("TRN2", target_bir_lowering=False)
    x_n = nc.dram_tensor("x_n", (_C, _H, _W), xdt, kind="ExternalInput")
    w_d = nc.dram_tensor("w_all", (128, 4 * _WPAIR), f16, kind="ExternalInput")
    out_d = nc.dram_tensor("out_d", (8, _C, 8, 256), f16, kind="ExternalOutput")

    def sub_ap(base_ap, extra_off, free_dims):
        return AP(base_ap.tensor, base_ap.offset + extra_off,
                  [list(base_ap.ap[0])] + [list(d) for d in free_dims])

    ROWLEN = _RW * span

    with TileContext(nc) as tc:
        with tc.tile_pool(name="fpool", bufs=3) as fpool, \
             tc.tile_pool(name="wpool", bufs=1) as wpool, \
             tc.tile_pool(name="tpool", bufs=2) as tpool, \
             tc.tile_pool(name="mpool", bufs=2) as mpool, \
             tc.tile_pool(name="opool", bufs=2) as opool, \
             tc.tile_pool(name="npool", bufs=2) as npool:
            W_sb = wpool.tile([128, 4 * _WPAIR], f16)
            nc.scalar.dma_start(out=W_sb[:], in_=w_d[:])
            wb = W_sb[:]

            def emit_load(pair):
                # per band: rows [r0, r0+20) x cols [c0, c0+span)
                F = fpool.tile([128, ROWLEN], xdt)
                for s in range(2):
                    src = AP(x_n[:].tensor, pair["r0"][s] * _W + pair["c0"][s],
                             [[_H * _W, _C], [_W, _RW], [1, span]])
                    nc.sync.dma_start(out=F[s * 64:(s + 1) * 64, :], in_=src)
                return F

            def emit_store(pair, O):
                for s in range(2):
                    b = pair["bands"][s]
                    dst = AP(out_d[:].tensor, b * (_C * 8 * 256),
                             [[8 * 256, _C], [1, 8 * 256]])
                    nc.scalar.dma_start(out=dst, in_=O[s * 64:(s + 1) * 64, :])

            PREFETCH = 3
            ftiles = {i: emit_load(pairs[i]) for i in range(PREFETCH)}
            pending_store = None
            for pi, pair in enumerate(pairs):
                p = pair["p"]
                slots = pair["slots"]
                bb = ftiles.pop(pi)[:]
                if pi + PREFETCH < len(pairs):
                    ftiles[pi + PREFETCH] = emit_load(pairs[pi + PREFETCH])

                ndve = _S1_DVE_SLOTS[p]
                e1a = nc.vector
                e2 = nc.vector

                wpair = p * _WPAIR
                T = tpool.tile([128, 8 * 288], f16)
                tb = T[:]
                rtmax = max(sl["rt"] for sl in slots)
                ctmax = max(sl["ct"] for sl in slots)
                Mts = {}
                for k in range(rtmax):
                    nk = sum(1 for sl in slots if sl["rt"] > k)
                    if k > 0:
                        Mts[k] = mpool.tile([128, 8 * 288], f16, name=f"Mt{k}")
                    for pos, sl in enumerate(slots):
                        if sl["rt"] <= k:
                            break
                        src = sub_ap(bb, (sl["rho0"] + k) * span + sl["sig0"],
                                     [[span, 16], [1, 18]])
                        w_ap = sub_ap(wb, wpair + pos * _WSLOT + k * 16,
                                      [[1, 16], [0, 18]])
                        dstt = tb if k == 0 else Mts[k][:]
                        dst = sub_ap(dstt, pos * 288, [[18, 16], [1, 18]])
                        e1 = nc.vector if pos < ndve else nc.gpsimd
                        e1.tensor_tensor(out=dst, in0=src, in1=w_ap, op=mult)
                    if k > 0:
                        e1a.tensor_tensor(
                            out=sub_ap(tb, 0, [[1, nk * 288]]),
                            in0=sub_ap(tb, 0, [[1, nk * 288]]),
                            in1=sub_ap(Mts[k][:], 0, [[1, nk * 288]]),
                            op=add)
                O = opool.tile([128, 8 * 256], f16)
                ob = O[:]
                for ik in range(ctmax):
                    nk = sum(1 for sl in slots if sl["ct"] > ik)
                    in0 = sub_ap(tb, ik, [[288, nk], [18, 16], [1, 16]])
                    w_ap = sub_ap(wb, wpair + _RT_MAX * 16 + ik * 16,
                                  [[_WSLOT, nk], [0, 16], [1, 16]])
                    if ik == 0:
                        o_ap = sub_ap(ob, 0, [[256, nk], [16, 16], [1, 16]])
                        e2.tensor_tensor(out=o_ap, in0=in0, in1=w_ap, op=mult)
                    else:
                        MO = npool.tile([128, 8 * 256], f16)
                        m_ap = sub_ap(MO[:], 0, [[256, nk], [16, 16], [1, 16]])
                        e2.tensor_tensor(out=m_ap, in0=in0, in1=w_ap, op=mult)
                        e2.tensor_tensor(
                            out=sub_ap(ob, 0, [[1, nk * 256]]),
                            in0=sub_ap(ob, 0, [[1, nk * 256]]),
                            in1=sub_ap(MO[:], 0, [[1, nk * 256]]),
                            op=add)
                if pending_store is not None:
                    emit_store(*pending_store)
                pending_store = (pair, O)

            emit_store(*pending_store)
    nc.compile()
    return nc


def _prepare(offset):
    pairs, span, w_all = _plan(offset)
    nc = _build(pairs, span, _XDT)
    aux = dict(pairs=pairs, span=span, w_all=w_all.astype(np.float16))
    return nc, aux


def _run(nc, x, aux, **kwargs):
    from concourse.bass_utils import run_bass_kernel_spmd
    xdt = np.float32 if _XDT == "f32" else np.float16
    in_maps = [{"x_n": np.ascontiguousarray(x[n]).astype(xdt, copy=False),
                "w_all": aux["w_all"]}
               for n in range(_N)]
    return run_bass_kernel_spmd(nc, in_maps, core_ids=list(range(_N)), **kwargs)


def _postprocess(out_dev, pairs):
    """out_dev: (8 bands, C, 8 slotpos, 256) fp16 -> (M, C, 16, 16) f32."""
    out = np.empty((_M, _C, _P, _P), dtype=np.float32)
    for p in range(4):
        order = [sl["mh"] for sl in pairs[p]["slots"]]
        for s in range(2):
            b = p + 4 * s
            for pos, mh in enumerate(order):
                # device slot layout is [j][i]-major (keeps DVE last dims
                # packed); semantic output is [i][j]
                out[mh * 8 + b] = out_dev[b, :, pos].astype(
                    np.float32).reshape(_C, _P, _P).transpose(0, 2, 1)
    return out


def kernel(x: np.ndarray, offset: np.ndarray) -> np.ndarray:
    x = np.asarray(x, dtype=np.float32)
    offset = np.asarray(offset, dtype=np.float32)
    nc, aux = _prepare(offset)
    res = _run(nc, x, aux)
    return np.stack([_postprocess(res.results[n]["out_d"], aux["pairs"])
                     for n in range(_N)])
